# revision 15
# baseline (speedup 1.0000x reference)
"""Trainium2 Bass kernel for nn_CrossPairMemory.

Sharding: data-parallel over batch across 8 NeuronCores (512 rows each),
weights replicated per core, no collectives.

Algebraic restructuring (all folds are weight-only, done host-side in fp32):
  * The fusion first Linear collapses through the associative memory read:
      h = [A_P | A_M] @ C,  C = [[vP @ W1_top + b1], [vM @ W1_bot]]
    where A_* are the (Bc, 64) attention matrices.  This removes the
    26 GFLOP/core (Bc,7168)x(7168,3584) matmul entirely.
  * LayerNorm-1 statistics come from the same algebra:
      sum_f h = c1^T a      with c1 = C.sum(axis=1)
      sum_f h^2 = a^T G a   with G = C @ C^T   (kept in fp32 on device)
    so h is never materialized pre-norm.
  * LayerNorm-1 apply is folded into the mm1 matmul: the attention matrix
    is scaled per-column by rstd, C is pre-scaled per-feature by ln_g, and
    the -mu*rstd*ln_g offset enters via a K=1 rank-1 matmul into the same
    PSUM accumulation; gelu(scale+bias) reads PSUM directly.
  * The second fusion Linear and the per-pair output Linear collapse:
      W2' = W2 @ blockdiag(pair_w[:,128:,:]),  b' = b2 @ blockdiag(..) + pair_b
    so one (Bc,3584)x(3584,3584) matmul plus a small pair_states @ pw_top
    term produces the pre-LN per-pair outputs directly, batch-major.

Input-adaptive fast paths (checked on the actual arrays, general fallback):
skip the final LN scale/shift when pair_ln_g==1 and pair_ln_b==0, and skip
the stage-C bias matmul when the folded bias is exactly zero.
"""

import sys

for _p in ("/opt/trn_rl_repo",):
    if _p not in sys.path:
        sys.path.insert(0, _p)

import numpy as np
import ml_dtypes

import concourse.bass as bass
import concourse.tile as tile
from concourse import bacc, mybir
from concourse import bass_utils

BF = ml_dtypes.bfloat16
dt = mybir.dt
AF = mybir.ActivationFunctionType
ALU = mybir.AluOpType

NCORES = 8
B, P, PD, MD, S = 4096, 28, 128, 256, 64
D = P * PD            # 3584
Bc = B // NCORES      # 512 batch rows per core
NBT = Bc // PD        # 4 batch tiles of 128
MG = 7                # mm2 column groups of 4 pairs (512 cols)
EPS = 1e-5


def _build(unit_ln2, zero_bias):
    nc = bacc.Bacc(
        "TRN2", target_bir_lowering=False, debug=False, num_devices=NCORES
    )

    def din(name, shape, dty):
        return nc.dram_tensor(name, list(shape), dty, kind="ExternalInput").ap()

    psT = din("psT", (P, PD, Bc), dt.bfloat16)      # pair_states^T per pair
    msT = din("msT", (2, PD, Bc), dt.bfloat16)      # macro_state^T, 2 tiles
    kP = din("kP", (PD, S), dt.bfloat16)            # pair keys^T, pre-scaled
    kM = din("kM", (2, PD, S), dt.bfloat16)         # macro keys^T, pre-scaled
    Cg = din("Cg", (PD, D), dt.bfloat16)            # C * ln1_g, slot-major
    c1 = din("c1", (PD, 1), dt.float32)             # C row-sums
    Gm = din("Gm", (PD, PD), dt.float32)            # C @ C^T
    grow = din("grow", (1, D), dt.bfloat16)         # ln1_g row
    be1t = din("be1t", (PD, P), dt.float32)         # ln1_b, feature-major
    w2p = din("w2p", (MG, PD, P, 4 * PD), dt.bfloat16)  # W2' blocks
    pwt = din("pwt", (PD, P, PD), dt.bfloat16)      # pair_w top half, d-major
    if not zero_bias:
        bprow = din("bprow", (1, D), dt.bfloat16)   # b2 @ pw_bot + pair_b
    if not unit_ln2:
        g2bc = din("g2bc", (PD, P, PD), dt.float32)  # pair_ln_g broadcast
        b2bc = din("b2bc", (PD, P, PD), dt.float32)  # pair_ln_b broadcast
    out = nc.dram_tensor(
        "out", [Bc, D], dt.float32, kind="ExternalOutput"
    ).ap()

    with tile.TileContext(nc) as tc:
        with (
            tc.tile_pool(name="const", bufs=1) as const,
            tc.tile_pool(name="res", bufs=1) as res,
            tc.tile_pool(name="gres", bufs=1) as gres,
            tc.tile_pool(name="w2s", bufs=2) as pw2,
        ):
            ones_col_b = const.tile([PD, 1], dt.bfloat16, tag="ocb", name="ocb")
            nc.vector.memset(ones_col_b, 1.0)
            ones_col_f = const.tile([PD, 1], dt.float32, tag="ocf", name="ocf")
            nc.vector.memset(ones_col_f, 1.0)
            ones_row_b = const.tile([1, PD], dt.bfloat16, tag="orb", name="orb")
            nc.vector.memset(ones_row_b, 1.0)
            ones_row_f = const.tile([1, PD], dt.float32, tag="orf", name="orf")
            nc.vector.memset(ones_row_f, 1.0)
            eps_t = const.tile([PD, 1], dt.float32, tag="eps", name="eps")
            nc.vector.memset(eps_t, EPS)
            warm = const.tile([PD, Bc], dt.bfloat16, tag="warm", name="warm")
            nc.vector.memset(warm, 0.0)

            cst = {}

            def cload(nm, src, shp, dty):
                t = const.tile(list(shp), dty, tag=nm, name=nm)
                nc.sync.dma_start(t, src)
                cst[nm] = t

            # DMAs in consumption order: scores path first, stage C last.
            cload("kP", kP, (PD, S), dt.bfloat16)
            kM_sb, ms_sb = [], []
            for i in range(2):
                t = const.tile([PD, S], dt.bfloat16, tag=f"kM{i}", name=f"kM{i}")
                nc.sync.dma_start(t, kM[i])
                kM_sb.append(t)
                t = const.tile([PD, Bc], dt.bfloat16, tag=f"ms{i}", name=f"ms{i}")
                nc.sync.dma_start(t, msT[i])
                ms_sb.append(t)
            psT_sb = []
            for p in range(P):
                t = res.tile([PD, Bc], dt.bfloat16, tag=f"psT{p}", name=f"psT{p}")
                nc.sync.dma_start(t, psT[p])
                psT_sb.append(t)
            cload("Cg", Cg, (PD, D), dt.bfloat16)
            cload("c1", c1, (PD, 1), dt.float32)
            cload("Gm", Gm, (PD, PD), dt.float32)
            cload("grow", grow, (1, D), dt.bfloat16)
            cload("be1t", be1t, (PD, P), dt.float32)
            # prefetch first two W2' blocks behind the front-critical loads
            w2tiles = {}
            for mg in range(2):
                t = pw2.tile([PD, P, 4 * PD], dt.bfloat16, tag="w2b", name="w2b")
                nc.sync.dma_start(t, w2p[mg])
                w2tiles[mg] = t
            cload("pwt", pwt, (PD, P, PD), dt.bfloat16)
            if not zero_bias:
                cload("bprow", bprow, (1, D), dt.bfloat16)
            if not unit_ln2:
                cload("g2bc", g2bc, (PD, P, PD), dt.float32)
                cload("b2bc", b2bc, (PD, P, PD), dt.float32)

            # post-gelu activations, feature-major k-tiles (mm2 stationary)
            gsb = [
                gres.tile([PD, Bc], dt.bfloat16, tag=f"g{n}", name=f"g{n}")
                for n in range(P)
            ]

            # ---------------- front: memory read + LN1 + gelu ----------
            with tc.tile_pool(name="fr", bufs=1) as fr:
                abPM = fr.tile([PD, Bc], dt.bfloat16, tag="abPM", name="abPM")
                abF = fr.tile([PD, Bc], dt.float32, tag="abF", name="abF")
                aprime = fr.tile([PD, Bc], dt.bfloat16, tag="apr", name="apr")
                negmr = fr.tile([1, Bc], dt.bfloat16, tag="negmr", name="negmr")

                with (
                    tc.tile_pool(name="psWm", bufs=2, space="PSUM") as ppwm,
                    tc.tile_pool(name="psSp", bufs=2, space="PSUM") as ppsp,
                    tc.tile_pool(name="psBc", bufs=2, space="PSUM") as ppbc,
                    tc.tile_pool(name="psRw", bufs=2, space="PSUM") as pprw,
                ):
                    # spin the PE p-state up while input DMAs stream
                    for _ in range(5):
                        wps = ppwm.tile([PD, Bc], dt.float32, tag="wps",
                                        name="wps")
                        nc.tensor.matmul(wps, warm[:, 0:PD], warm,
                                         start=True, stop=True)

                    spP = ppsp.tile([S, Bc], dt.float32, tag="sp", name="spP")
                    for p in range(P):
                        nc.tensor.matmul(spP, cst["kP"], psT_sb[p],
                                         start=(p == 0), stop=(p == P - 1))
                    ebP = fr.tile([S, Bc], dt.bfloat16, tag="ebP", name="ebP")
                    nc.scalar.activation(ebP, spP, AF.Exp)
                    spM = ppsp.tile([S, Bc], dt.float32, tag="sp", name="spM")
                    nc.tensor.matmul(spM, kM_sb[0], ms_sb[0],
                                     start=True, stop=False)
                    nc.tensor.matmul(spM, kM_sb[1], ms_sb[1],
                                     start=False, stop=True)
                    ebM = fr.tile([S, Bc], dt.bfloat16, tag="ebM", name="ebM")
                    nc.scalar.activation(ebM, spM, AF.Exp)
                    denP = pprw.tile([1, Bc], dt.float32, tag="den", name="denP")
                    nc.tensor.matmul(denP, ones_col_b[0:S, :], ebP,
                                     start=True, stop=True)
                    denM = pprw.tile([1, Bc], dt.float32, tag="den", name="denM")
                    nc.tensor.matmul(denM, ones_col_b[0:S, :], ebM,
                                     start=True, stop=True)
                    rrP = fr.tile([1, Bc], dt.float32, tag="rrP", name="rrP")
                    nc.vector.reciprocal(rrP, denP)
                    rrM = fr.tile([1, Bc], dt.float32, tag="rrM", name="rrM")
                    nc.vector.reciprocal(rrM, denM)
                    rbcP = ppbc.tile([S, Bc], dt.float32, tag="rbc", name="rbcP")
                    nc.tensor.matmul(rbcP, ones_row_f[:, 0:S], rrP,
                                     start=True, stop=True)
                    rbcM = ppbc.tile([S, Bc], dt.float32, tag="rbc", name="rbcM")
                    nc.tensor.matmul(rbcM, ones_row_f[:, 0:S], rrM,
                                     start=True, stop=True)
                    nc.vector.tensor_mul(abPM[0:S, :], ebP, rbcP)
                    nc.vector.tensor_mul(abPM[S:2 * S, :], ebM, rbcM)

                # f32 copy of attention for exact fp32 stats matmuls
                nc.scalar.activation(abF, abPM, AF.Copy)

                with tc.tile_pool(name="psSt", bufs=1, space="PSUM") as ppst:
                    murow = ppst.tile([1, Bc], dt.float32, tag="mu", name="mu")
                    nc.tensor.matmul(murow, cst["c1"], abF, start=True, stop=True)
                    Gt = ppst.tile([PD, Bc], dt.float32, tag="Gt", name="Gt")
                    nc.tensor.matmul(Gt, cst["Gm"], abF, start=True, stop=True)
                    qq = fr.tile([PD, Bc], dt.float32, tag="qq", name="qq")
                    nc.vector.tensor_mul(qq, abF, Gt)
                    sqrow = ppst.tile([1, Bc], dt.float32, tag="sq", name="sq")
                    nc.tensor.matmul(sqrow, ones_col_f, qq, start=True, stop=True)

                    m = fr.tile([1, Bc], dt.float32, tag="m", name="m")
                    nc.vector.tensor_scalar_mul(m, murow, 1.0 / D)
                    m2 = fr.tile([1, Bc], dt.float32, tag="m2", name="m2")
                    nc.vector.tensor_mul(m2, m, m)
                    var = fr.tile([1, Bc], dt.float32, tag="var", name="var")
                    nc.vector.scalar_tensor_tensor(
                        var, sqrow, 1.0 / D, m2, op0=ALU.mult, op1=ALU.subtract
                    )
                    sd = fr.tile([1, Bc], dt.float32, tag="sd", name="sd")
                    nc.scalar.activation(sd, var, AF.Sqrt,
                                         bias=eps_t[0:1, :], scale=1.0)
                    rstd = fr.tile([1, Bc], dt.float32, tag="rstd", name="rstd")
                    nc.vector.reciprocal(rstd, sd)
                    # negmr = -mu * rstd (bf16 row, K=1 matmul operand)
                    nc.vector.scalar_tensor_tensor(
                        negmr, m, -1.0, rstd, op0=ALU.mult, op1=ALU.mult
                    )
                    rstd_bc = ppst.tile([PD, Bc], dt.float32, tag="rbc2",
                                        name="rbc2")
                    nc.tensor.matmul(rstd_bc, ones_row_f, rstd,
                                     start=True, stop=True)
                    nc.vector.tensor_mul(aprime, abPM, rstd_bc)

                # ---- mm1' + stage BC, software-pipelined ----
                # mm1' per tile n: C-matmul, then the -mu*rstd*g offset is a
                # DVE add of a broadcast PSUM tile (off the PE critical path),
                # then gelu reads PSUM.  The first PRE stage-C accumulations
                # interleave k=n matmuls two steps behind gsb production, so
                # stage C starts while the front is still finishing.
                PRE, LAG = 3, 2
                iters = [(mg, bt) for mg in range(MG) for bt in range(NBT)]
                with (
                    tc.tile_pool(name="psM1", bufs=2, space="PSUM") as ppm1,
                    tc.tile_pool(name="psOf", bufs=1, space="PSUM") as ppof,
                    tc.tile_pool(name="yo", bufs=3) as pyo,
                    tc.tile_pool(name="sc", bufs=4) as psc,
                    tc.tile_pool(name="psC", bufs=3, space="PSUM") as ppc,
                ):

                    def emit_po_mm(po, mg, bt, k):
                        bs = slice(bt * PD, (bt + 1) * PD)
                        first = (k == 0) if zero_bias else (k == -1)
                        if k == -1:
                            if not zero_bias:
                                nc.tensor.matmul(
                                    po, ones_row_b,
                                    cst["bprow"][:, mg * 4 * PD:
                                                 (mg + 1) * 4 * PD],
                                    start=True, stop=False)
                            return
                        nc.tensor.matmul(
                            po, gsb[k][:, bs], w2b_cur[0][:, k, :],
                            start=first, stop=(k == P - 1),
                            skip_group_check=not first,
                        )
                        if k == 0:
                            for s in range(4):
                                pidx = 4 * mg + s
                                nc.tensor.matmul(
                                    po[:, s * PD:(s + 1) * PD],
                                    psT_sb[pidx][:, bs],
                                    cst["pwt"][:, pidx, :],
                                    start=False, stop=False,
                                    skip_group_check=True,
                                )

                    def emit_ln2_dma(po, mg, bt):
                        bs = slice(bt * PD, (bt + 1) * PD)
                        mgsl = slice(mg * 4 * PD, (mg + 1) * 4 * PD)
                        y4 = pyo.tile([PD, 4 * PD], dt.float32, tag="y4",
                                      name="y4")
                        mva = psc.tile([PD, 8], dt.float32, tag="mv", name="mv")
                        rst4 = psc.tile([PD, 4], dt.float32, tag="rst4",
                                        name="rst4")
                        for s in range(4):
                            st6 = psc.tile([PD, 6], dt.float32, tag="st6",
                                           name="st6")
                            nc.vector.bn_stats(st6, po[:, s * PD:(s + 1) * PD])
                            nc.vector.bn_aggr(mva[:, 2 * s:2 * s + 2], st6)
                            sd2 = psc.tile([PD, 1], dt.float32, tag="sd2",
                                           name="sd2")
                            nc.scalar.activation(
                                sd2, mva[:, 2 * s + 1:2 * s + 2], AF.Sqrt,
                                bias=eps_t, scale=1.0)
                            nc.vector.reciprocal(rst4[:, s:s + 1], sd2)
                        for s in range(4):
                            pidx = 4 * mg + s
                            ssl = slice(s * PD, (s + 1) * PD)
                            if unit_ln2:
                                nc.vector.tensor_scalar(
                                    y4[:, ssl], po[:, ssl],
                                    mva[:, 2 * s:2 * s + 1], rst4[:, s:s + 1],
                                    op0=ALU.subtract, op1=ALU.mult,
                                )
                            else:
                                tn = psc.tile([PD, PD], dt.float32, tag="tn",
                                              name="tn")
                                nc.vector.tensor_scalar(
                                    tn, po[:, ssl],
                                    mva[:, 2 * s:2 * s + 1], rst4[:, s:s + 1],
                                    op0=ALU.subtract, op1=ALU.mult,
                                )
                                tg = psc.tile([PD, PD], dt.float32, tag="tg",
                                              name="tg")
                                nc.vector.tensor_mul(
                                    tg, tn, cst["g2bc"][:, pidx, :])
                                nc.vector.tensor_add(
                                    y4[:, ssl], tg, cst["b2bc"][:, pidx, :])
                        nc.sync.dma_start(out[bs, mgsl], y4)

                    w2b_cur = [w2tiles.pop(0)]
                    pre_pos = [
                        ppc.tile([PD, 4 * PD], dt.float32, tag="po",
                                 name=f"po_pre{i}")
                        for i in range(PRE)
                    ]
                    # mm1' with lagged pre-po matmul streams
                    for n in range(P):
                        nsl = slice(n * PD, (n + 1) * PD)
                        pm = ppm1.tile([PD, Bc], dt.float32, tag="pm", name="pm")
                        nc.tensor.matmul(pm, cst["grow"][:, nsl], negmr,
                                         start=True, stop=False)
                        nc.tensor.matmul(pm, cst["Cg"][:, nsl], aprime,
                                         start=False, stop=True)
                        nc.scalar.activation(
                            gsb[n], pm, AF.Gelu,
                            bias=cst["be1t"][:, n:n + 1], scale=1.0,
                        )
                        k = n - LAG
                        if k == 0:
                            for i in range(PRE):
                                emit_po_mm(pre_pos[i], *iters[i], -1)
                        if k >= 0:
                            for i in range(PRE):
                                emit_po_mm(pre_pos[i], *iters[i], k)
                    for k in range(P - LAG, P):
                        for i in range(PRE):
                            emit_po_mm(pre_pos[i], *iters[i], k)
                    for i in range(PRE):
                        emit_ln2_dma(pre_pos[i], *iters[i])

                    # remaining iterations
                    for idx in range(PRE, len(iters)):
                        mg, bt = iters[idx]
                        if bt == 0 and mg > 0:
                            w2b_cur[0] = w2tiles.pop(mg)
                        po = ppc.tile([PD, 4 * PD], dt.float32, tag="po",
                                      name="po")
                        emit_po_mm(po, mg, bt, -1)
                        for k in range(P):
                            emit_po_mm(po, mg, bt, k)
                        emit_ln2_dma(po, mg, bt)
                        if bt == NBT - 1 and mg + 2 < MG:
                            t = pw2.tile([PD, P, 4 * PD], dt.bfloat16,
                                         tag="w2b", name="w2b")
                            nc.sync.dma_start(t, w2p[mg + 2])
                            w2tiles[mg + 2] = t

    nc.compile()
    return nc


_CACHE = {}


def _get_nc(unit_ln2, zero_bias):
    key = (unit_ln2, zero_bias)
    if key not in _CACHE:
        _CACHE[key] = _build(unit_ln2, zero_bias)
    return _CACHE[key]


def _prep_in_maps(inputs):
    f32 = np.float32
    g = lambda k: np.asarray(inputs[k], f32)

    psT_full = np.asarray(g("pair_states").transpose(1, 2, 0), dtype=BF)  # [P,PD,B]
    msT_full = np.asarray(g("macro_state").T, dtype=BF)                   # [MD,B]

    W1 = g("fusion_w1")                       # (7168, 3584)
    C = np.concatenate(
        [
            g("mem_pair_vals") @ W1[:D] + g("fusion_b1")[None, :],
            g("mem_macro_vals") @ W1[D:],
        ],
        axis=0,
    )                                          # (128, 3584)
    g1 = g("fusion_ln_g")
    pw = g("pair_w")                           # (28, 256, 128)
    pwA, pwB = pw[:, :PD, :], pw[:, PD:, :]
    # W2' = W2 @ blockdiag(pwB): (3584, 28, 128)
    W2r = g("fusion_w2").reshape(D, P, PD)
    W2p = np.matmul(W2r.transpose(1, 0, 2), pwB)          # (28, 3584, 128)
    W2p = W2p.transpose(1, 0, 2).reshape(D, D)
    bp = (
        np.einsum("pc,pce->pe", g("fusion_b2").reshape(P, PD), pwB)
        + g("pair_b")
    ).reshape(1, D)

    import os
    ln2g, ln2b = g("pair_ln_g"), g("pair_ln_b")
    unit_ln2 = bool((ln2g == 1.0).all() and (ln2b == 0.0).all())
    zero_bias = bool((bp == 0.0).all())
    if os.environ.get("K_NOFAST"):
        unit_ln2 = zero_bias = False

    shared = {
        "kP": np.ascontiguousarray(
            (g("mem_pair_keys").T / (P * np.sqrt(PD))).astype(BF)),
        "kM": np.ascontiguousarray(
            (g("mem_macro_keys").T / np.sqrt(MD)).reshape(2, PD, S).astype(BF)),
        "Cg": np.ascontiguousarray((C * g1[None, :]).astype(BF)),
        "c1": np.ascontiguousarray(C.sum(axis=1, dtype=np.float64)
                                   .astype(f32).reshape(PD, 1)),
        "Gm": np.ascontiguousarray((C @ C.T).astype(f32)),
        "grow": np.ascontiguousarray(g1.reshape(1, D).astype(BF)),
        "be1t": np.ascontiguousarray(g("fusion_ln_b").reshape(P, PD).T),
        "w2p": np.ascontiguousarray(
            W2p.reshape(P, PD, MG, 4 * PD).transpose(2, 1, 0, 3).astype(BF)),
        "pwt": np.ascontiguousarray(pwA.transpose(1, 0, 2).astype(BF)),
    }
    if not zero_bias:
        shared["bprow"] = np.ascontiguousarray(bp.astype(BF))
    if not unit_ln2:
        shared["g2bc"] = np.ascontiguousarray(
            np.broadcast_to(ln2g[None], (PD, P, PD)))
        shared["b2bc"] = np.ascontiguousarray(
            np.broadcast_to(ln2b[None], (PD, P, PD)))
    in_maps = []
    for c in range(NCORES):
        m = dict(shared)
        m["psT"] = np.ascontiguousarray(psT_full[:, :, c * Bc:(c + 1) * Bc])
        m["msT"] = np.ascontiguousarray(
            msT_full[:, c * Bc:(c + 1) * Bc].reshape(2, PD, Bc))
        in_maps.append(m)
    return in_maps, unit_ln2, zero_bias


def _run(inputs, trace=False):
    in_maps, unit_ln2, zero_bias = _prep_in_maps(inputs)
    nc = _get_nc(unit_ln2, zero_bias)
    res = bass_utils.run_bass_kernel_spmd(
        nc, in_maps, core_ids=list(range(NCORES)), trace=trace
    )
    outp = np.concatenate(
        [res.results[c]["out"] for c in range(NCORES)], axis=0
    ).reshape(B, P, PD)
    return np.ascontiguousarray(outp.astype(np.float32)), res


def kernel(**inputs):
    outp, _ = _run(inputs, trace=False)
    return outp


# revision 17
# speedup vs baseline: 1.0392x; 1.0392x over previous
"""Trainium2 Bass kernel for nn_CrossPairMemory.

Sharding: data-parallel over batch across 8 NeuronCores (512 rows each),
weights replicated per core, no collectives.

Algebraic restructuring (all folds are weight-only, done host-side in fp32):
  * The fusion first Linear collapses through the associative memory read:
      h = [A_P | A_M] @ C,  C = [[vP @ W1_top + b1], [vM @ W1_bot]]
    where A_* are the (Bc, 64) attention matrices.  This removes the
    26 GFLOP/core (Bc,7168)x(7168,3584) matmul entirely.
  * LayerNorm-1 statistics come from the same algebra:
      sum_f h = c1^T a      with c1 = C.sum(axis=1)
      sum_f h^2 = a^T G a   with G = C @ C^T   (kept in fp32 on device)
    so h is never materialized pre-norm.
  * LayerNorm-1 apply is folded into the mm1 matmul: the attention matrix
    is scaled per-column by rstd, C is pre-scaled per-feature by ln_g, and
    the -mu*rstd*ln_g offset enters via a K=1 rank-1 matmul into the same
    PSUM accumulation; gelu(scale+bias) reads PSUM directly.
  * The second fusion Linear and the per-pair output Linear collapse:
      W2' = W2 @ blockdiag(pair_w[:,128:,:]),  b' = b2 @ blockdiag(..) + pair_b
    so one (Bc,3584)x(3584,3584) matmul plus a small pair_states @ pw_top
    term produces the pre-LN per-pair outputs directly, batch-major.

Input-adaptive fast paths (checked on the actual arrays, general fallback):
skip the final LN scale/shift when pair_ln_g==1 and pair_ln_b==0, and skip
the stage-C bias matmul when the folded bias is exactly zero.
"""

import sys

for _p in ("/opt/trn_rl_repo",):
    if _p not in sys.path:
        sys.path.insert(0, _p)

import numpy as np
import ml_dtypes

import concourse.bass as bass
import concourse.tile as tile
from concourse import bacc, mybir
from concourse import bass_utils

BF = ml_dtypes.bfloat16
dt = mybir.dt
AF = mybir.ActivationFunctionType
ALU = mybir.AluOpType

NCORES = 8
B, P, PD, MD, S = 4096, 28, 128, 256, 64
D = P * PD            # 3584
Bc = B // NCORES      # 512 batch rows per core
NBT = Bc // PD        # 4 batch tiles of 128
MG = 7                # mm2 column groups of 4 pairs (512 cols)
EPS = 1e-5


def _build(unit_ln2, zero_bias):
    nc = bacc.Bacc(
        "TRN2", target_bir_lowering=False, debug=False, num_devices=NCORES
    )

    def din(name, shape, dty):
        return nc.dram_tensor(name, list(shape), dty, kind="ExternalInput").ap()

    psT = din("psT", (P, PD, Bc), dt.bfloat16)      # pair_states^T per pair
    msT = din("msT", (2, PD, Bc), dt.bfloat16)      # macro_state^T, 2 tiles
    kP = din("kP", (PD, S), dt.bfloat16)            # pair keys^T, pre-scaled
    kM = din("kM", (2, PD, S), dt.bfloat16)         # macro keys^T, pre-scaled
    Cg = din("Cg", (PD, D), dt.bfloat16)            # C * ln1_g, slot-major
    c1 = din("c1", (PD, 1), dt.bfloat16)            # C row-sums
    Gm = din("Gm", (PD, PD), dt.float32)            # C @ C^T
    grow = din("grow", (1, D), dt.bfloat16)         # ln1_g row
    be1t = din("be1t", (PD, P), dt.float32)         # ln1_b, feature-major
    w2p = din("w2p", (MG, PD, P, 4 * PD), dt.bfloat16)  # W2' blocks
    pwt = din("pwt", (PD, P, PD), dt.bfloat16)      # pair_w top half, d-major
    if not zero_bias:
        bprow = din("bprow", (1, D), dt.bfloat16)   # b2 @ pw_bot + pair_b
    if not unit_ln2:
        g2bc = din("g2bc", (PD, P, PD), dt.float32)  # pair_ln_g broadcast
        b2bc = din("b2bc", (PD, P, PD), dt.float32)  # pair_ln_b broadcast
    out = nc.dram_tensor(
        "out", [Bc, D], dt.float32, kind="ExternalOutput"
    ).ap()

    with tile.TileContext(nc) as tc:
        with (
            tc.tile_pool(name="const", bufs=1) as const,
            tc.tile_pool(name="res", bufs=1) as res,
            tc.tile_pool(name="gres", bufs=1) as gres,
            tc.tile_pool(name="w2s", bufs=2) as pw2,
        ):
            ones_col_b = const.tile([PD, 1], dt.bfloat16, tag="ocb", name="ocb")
            nc.vector.memset(ones_col_b, 1.0)
            ones_col_f = const.tile([PD, 1], dt.float32, tag="ocf", name="ocf")
            nc.vector.memset(ones_col_f, 1.0)
            ones_row_b = const.tile([1, PD], dt.bfloat16, tag="orb", name="orb")
            nc.vector.memset(ones_row_b, 1.0)
            ones_row_f = const.tile([1, PD], dt.float32, tag="orf", name="orf")
            nc.vector.memset(ones_row_f, 1.0)
            eps_t = const.tile([PD, 1], dt.float32, tag="eps", name="eps")
            nc.vector.memset(eps_t, EPS)
            warm = const.tile([PD, Bc], dt.bfloat16, tag="warm", name="warm")
            nc.vector.memset(warm, 0.0)

            cst = {}

            def cload(nm, src, shp, dty):
                t = const.tile(list(shp), dty, tag=nm, name=nm)
                nc.sync.dma_start(t, src)
                cst[nm] = t

            # DMAs in consumption order: scores path first, stage C last.
            cload("kP", kP, (PD, S), dt.bfloat16)
            kM_sb, ms_sb = [], []
            for i in range(2):
                t = const.tile([PD, S], dt.bfloat16, tag=f"kM{i}", name=f"kM{i}")
                nc.sync.dma_start(t, kM[i])
                kM_sb.append(t)
                t = const.tile([PD, Bc], dt.bfloat16, tag=f"ms{i}", name=f"ms{i}")
                nc.sync.dma_start(t, msT[i])
                ms_sb.append(t)
            psT_sb = []
            for p in range(P):
                t = res.tile([PD, Bc], dt.bfloat16, tag=f"psT{p}", name=f"psT{p}")
                nc.sync.dma_start(t, psT[p])
                psT_sb.append(t)
            cload("Cg", Cg, (PD, D), dt.bfloat16)
            cload("c1", c1, (PD, 1), dt.bfloat16)
            cload("Gm", Gm, (PD, PD), dt.bfloat16)
            cload("grow", grow, (1, D), dt.bfloat16)
            cload("be1t", be1t, (PD, P), dt.float32)
            # prefetch first two W2' blocks behind the front-critical loads
            w2tiles = {}
            for mg in range(2):
                t = pw2.tile([PD, P, 4 * PD], dt.bfloat16, tag="w2b", name="w2b")
                nc.sync.dma_start(t, w2p[mg])
                w2tiles[mg] = t
            cload("pwt", pwt, (PD, P, PD), dt.bfloat16)
            if not zero_bias:
                cload("bprow", bprow, (1, D), dt.bfloat16)
            if not unit_ln2:
                cload("g2bc", g2bc, (PD, P, PD), dt.float32)
                cload("b2bc", b2bc, (PD, P, PD), dt.float32)

            # post-gelu activations, feature-major k-tiles (mm2 stationary)
            gsb = [
                gres.tile([PD, Bc], dt.bfloat16, tag=f"g{n}", name=f"g{n}")
                for n in range(P)
            ]

            # ---------------- front: memory read + LN1 + gelu ----------
            with tc.tile_pool(name="fr", bufs=1) as fr:
                abPM = fr.tile([PD, Bc], dt.bfloat16, tag="abPM", name="abPM")
                aprime = fr.tile([PD, Bc], dt.bfloat16, tag="apr", name="apr")
                negmr = fr.tile([1, Bc], dt.bfloat16, tag="negmr", name="negmr")

                with (
                    tc.tile_pool(name="psWm", bufs=2, space="PSUM") as ppwm,
                    tc.tile_pool(name="psSp", bufs=2, space="PSUM") as ppsp,
                    tc.tile_pool(name="psBc", bufs=2, space="PSUM") as ppbc,
                    tc.tile_pool(name="psRw", bufs=2, space="PSUM") as pprw,
                ):
                    # spin the PE p-state up while input DMAs stream
                    for _ in range(12):
                        wps = ppwm.tile([PD, Bc], dt.float32, tag="wps",
                                        name="wps")
                        nc.tensor.matmul(wps, warm[:, 0:PD], warm,
                                         start=True, stop=True)

                    spP = ppsp.tile([S, Bc], dt.float32, tag="sp", name="spP")
                    for p in range(P):
                        nc.tensor.matmul(spP, cst["kP"], psT_sb[p],
                                         start=(p == 0), stop=(p == P - 1))
                    ebP = fr.tile([S, Bc], dt.bfloat16, tag="ebP", name="ebP")
                    nc.scalar.activation(ebP, spP, AF.Exp)
                    spM = ppsp.tile([S, Bc], dt.float32, tag="sp", name="spM")
                    nc.tensor.matmul(spM, kM_sb[0], ms_sb[0],
                                     start=True, stop=False)
                    nc.tensor.matmul(spM, kM_sb[1], ms_sb[1],
                                     start=False, stop=True)
                    ebM = fr.tile([S, Bc], dt.bfloat16, tag="ebM", name="ebM")
                    nc.scalar.activation(ebM, spM, AF.Exp)
                    denP = pprw.tile([1, Bc], dt.float32, tag="den", name="denP")
                    nc.tensor.matmul(denP, ones_col_b[0:S, :], ebP,
                                     start=True, stop=True)
                    denM = pprw.tile([1, Bc], dt.float32, tag="den", name="denM")
                    nc.tensor.matmul(denM, ones_col_b[0:S, :], ebM,
                                     start=True, stop=True)
                    rrP = fr.tile([1, Bc], dt.float32, tag="rrP", name="rrP")
                    nc.vector.reciprocal(rrP, denP)
                    rrM = fr.tile([1, Bc], dt.float32, tag="rrM", name="rrM")
                    nc.vector.reciprocal(rrM, denM)
                    rbcP = ppbc.tile([S, Bc], dt.float32, tag="rbc", name="rbcP")
                    nc.tensor.matmul(rbcP, ones_row_f[:, 0:S], rrP,
                                     start=True, stop=True)
                    rbcM = ppbc.tile([S, Bc], dt.float32, tag="rbc", name="rbcM")
                    nc.tensor.matmul(rbcM, ones_row_f[:, 0:S], rrM,
                                     start=True, stop=True)
                    nc.vector.tensor_mul(abPM[0:S, :], ebP, rbcP)
                    nc.vector.tensor_mul(abPM[S:2 * S, :], ebM, rbcM)

                with tc.tile_pool(name="psSt", bufs=1, space="PSUM") as ppst:
                    murow = ppst.tile([1, Bc], dt.float32, tag="mu", name="mu")
                    nc.tensor.matmul(murow, cst["c1"], abPM, start=True, stop=True)
                    Gt = ppst.tile([PD, Bc], dt.float32, tag="Gt", name="Gt")
                    nc.tensor.matmul(Gt, cst["Gm"], abPM, start=True, stop=True)
                    qq = fr.tile([PD, Bc], dt.bfloat16, tag="qq", name="qq")
                    nc.vector.tensor_mul(qq, abPM, Gt)
                    sqrow = ppst.tile([1, Bc], dt.float32, tag="sq", name="sq")
                    nc.tensor.matmul(sqrow, ones_col_b, qq, start=True, stop=True)

                    m = fr.tile([1, Bc], dt.float32, tag="m", name="m")
                    nc.vector.tensor_scalar_mul(m, murow, 1.0 / D)
                    m2 = fr.tile([1, Bc], dt.float32, tag="m2", name="m2")
                    nc.vector.tensor_mul(m2, m, m)
                    var = fr.tile([1, Bc], dt.float32, tag="var", name="var")
                    nc.vector.scalar_tensor_tensor(
                        var, sqrow, 1.0 / D, m2, op0=ALU.mult, op1=ALU.subtract
                    )
                    sd = fr.tile([1, Bc], dt.float32, tag="sd", name="sd")
                    nc.scalar.activation(sd, var, AF.Sqrt,
                                         bias=eps_t[0:1, :], scale=1.0)
                    rstd = fr.tile([1, Bc], dt.float32, tag="rstd", name="rstd")
                    nc.vector.reciprocal(rstd, sd)
                    # negmr = -mu * rstd (bf16 row, K=1 matmul operand)
                    nc.vector.scalar_tensor_tensor(
                        negmr, m, -1.0, rstd, op0=ALU.mult, op1=ALU.mult
                    )
                    rstd_bc = ppst.tile([PD, Bc], dt.float32, tag="rbc2",
                                        name="rbc2")
                    nc.tensor.matmul(rstd_bc, ones_row_f, rstd,
                                     start=True, stop=True)
                    nc.vector.tensor_mul(aprime, abPM, rstd_bc)

                # ---- mm1' + stage BC, software-pipelined ----
                # mm1' per tile n: C-matmul, then the -mu*rstd*g offset is a
                # DVE add of a broadcast PSUM tile (off the PE critical path),
                # then gelu reads PSUM.  The first PRE stage-C accumulations
                # interleave k=n matmuls two steps behind gsb production, so
                # stage C starts while the front is still finishing.
                iters = [(mg, bt) for mg in range(MG) for bt in range(NBT)]
                with (
                    tc.tile_pool(name="psM1", bufs=2, space="PSUM") as ppm1,
                    tc.tile_pool(name="psOf", bufs=1, space="PSUM") as ppof,
                    tc.tile_pool(name="yo", bufs=3) as pyo,
                    tc.tile_pool(name="sc", bufs=4) as psc,
                    tc.tile_pool(name="psC", bufs=3, space="PSUM") as ppc,
                ):

                    def emit_po_mm(po, mg, bt, k):
                        bs = slice(bt * PD, (bt + 1) * PD)
                        first = (k == 0) if zero_bias else (k == -1)
                        if k == -1:
                            if not zero_bias:
                                nc.tensor.matmul(
                                    po, ones_row_b,
                                    cst["bprow"][:, mg * 4 * PD:
                                                 (mg + 1) * 4 * PD],
                                    start=True, stop=False)
                            return
                        nc.tensor.matmul(
                            po, gsb[k][:, bs], w2b_cur[0][:, k, :],
                            start=first, stop=(k == P - 1),
                            skip_group_check=not first,
                        )
                        if k == 0:
                            for s in range(4):
                                pidx = 4 * mg + s
                                nc.tensor.matmul(
                                    po[:, s * PD:(s + 1) * PD],
                                    psT_sb[pidx][:, bs],
                                    cst["pwt"][:, pidx, :],
                                    start=False, stop=False,
                                    skip_group_check=True,
                                )

                    def emit_ln2_dma(po, mg, bt):
                        bs = slice(bt * PD, (bt + 1) * PD)
                        mgsl = slice(mg * 4 * PD, (mg + 1) * 4 * PD)
                        y4 = pyo.tile([PD, 4 * PD], dt.float32, tag="y4",
                                      name="y4")
                        mva = psc.tile([PD, 8], dt.float32, tag="mv", name="mv")
                        rst4 = psc.tile([PD, 4], dt.float32, tag="rst4",
                                        name="rst4")
                        for s in range(4):
                            st6 = psc.tile([PD, 6], dt.float32, tag="st6",
                                           name="st6")
                            nc.vector.bn_stats(st6, po[:, s * PD:(s + 1) * PD])
                            nc.vector.bn_aggr(mva[:, 2 * s:2 * s + 2], st6)
                            sd2 = psc.tile([PD, 1], dt.float32, tag="sd2",
                                           name="sd2")
                            nc.scalar.activation(
                                sd2, mva[:, 2 * s + 1:2 * s + 2], AF.Sqrt,
                                bias=eps_t, scale=1.0)
                            nc.vector.reciprocal(rst4[:, s:s + 1], sd2)
                        for s in range(4):
                            pidx = 4 * mg + s
                            ssl = slice(s * PD, (s + 1) * PD)
                            if unit_ln2:
                                nc.vector.tensor_scalar(
                                    y4[:, ssl], po[:, ssl],
                                    mva[:, 2 * s:2 * s + 1], rst4[:, s:s + 1],
                                    op0=ALU.subtract, op1=ALU.mult,
                                )
                            else:
                                tn = psc.tile([PD, PD], dt.float32, tag="tn",
                                              name="tn")
                                nc.vector.tensor_scalar(
                                    tn, po[:, ssl],
                                    mva[:, 2 * s:2 * s + 1], rst4[:, s:s + 1],
                                    op0=ALU.subtract, op1=ALU.mult,
                                )
                                tg = psc.tile([PD, PD], dt.float32, tag="tg",
                                              name="tg")
                                nc.vector.tensor_mul(
                                    tg, tn, cst["g2bc"][:, pidx, :])
                                nc.vector.tensor_add(
                                    y4[:, ssl], tg, cst["b2bc"][:, pidx, :])
                        nc.sync.dma_start(out[bs, mgsl], y4)

                    def emit_half(mg, bt, h):
                        bs = slice(bt * PD, (bt + 1) * PD)
                        hsl = slice(h * 2 * PD, (h + 1) * 2 * PD)
                        po = ppc.tile([PD, 2 * PD], dt.float32, tag="poh",
                                      name=f"poh{h}")
                        if not zero_bias:
                            nc.tensor.matmul(
                                po, ones_row_b,
                                cst["bprow"][:, mg * 4 * PD + h * 2 * PD:
                                             mg * 4 * PD + (h + 1) * 2 * PD],
                                start=True, stop=False)
                        for k in range(P):
                            first = zero_bias and k == 0
                            nc.tensor.matmul(
                                po, gsb[k][:, bs], w2b_cur[0][:, k, hsl],
                                start=first, stop=(k == P - 1),
                                skip_group_check=not first)
                            if k == 0:
                                for s in range(2):
                                    pidx = 4 * mg + 2 * h + s
                                    nc.tensor.matmul(
                                        po[:, s * PD:(s + 1) * PD],
                                        psT_sb[pidx][:, bs],
                                        cst["pwt"][:, pidx, :],
                                        start=False, stop=False,
                                        skip_group_check=True)
                        y2 = pyo.tile([PD, 2 * PD], dt.float32, tag="y2",
                                      name=f"y2{h}")
                        mv2 = psc.tile([PD, 4], dt.float32, tag="mv2",
                                       name="mv2")
                        rs2 = psc.tile([PD, 2], dt.float32, tag="rs2",
                                       name="rs2")
                        for s in range(2):
                            pidx = 4 * mg + 2 * h + s
                            ssl = slice(s * PD, (s + 1) * PD)
                            st6 = psc.tile([PD, 6], dt.float32, tag="st6",
                                           name="st6")
                            nc.vector.bn_stats(st6, po[:, ssl])
                            nc.vector.bn_aggr(mv2[:, 2 * s:2 * s + 2], st6)
                            sd2 = psc.tile([PD, 1], dt.float32, tag="sd2",
                                           name="sd2")
                            nc.scalar.activation(
                                sd2, mv2[:, 2 * s + 1:2 * s + 2], AF.Sqrt,
                                bias=eps_t, scale=1.0)
                            nc.vector.reciprocal(rs2[:, s:s + 1], sd2)
                            if unit_ln2:
                                nc.vector.tensor_scalar(
                                    y2[:, ssl], po[:, ssl],
                                    mv2[:, 2 * s:2 * s + 1], rs2[:, s:s + 1],
                                    op0=ALU.subtract, op1=ALU.mult)
                            else:
                                tn = psc.tile([PD, PD], dt.float32, tag="tn",
                                              name="tn")
                                nc.vector.tensor_scalar(
                                    tn, po[:, ssl],
                                    mv2[:, 2 * s:2 * s + 1], rs2[:, s:s + 1],
                                    op0=ALU.subtract, op1=ALU.mult)
                                tg = psc.tile([PD, PD], dt.float32, tag="tg",
                                              name="tg")
                                nc.vector.tensor_mul(
                                    tg, tn, cst["g2bc"][:, pidx, :])
                                nc.vector.tensor_add(
                                    y2[:, ssl], tg, cst["b2bc"][:, pidx, :])
                        nc.sync.dma_start(
                            out[bs, mg * 4 * PD + h * 2 * PD:
                                mg * 4 * PD + (h + 1) * 2 * PD], y2)

                    w2b_cur = [None]
                    # mm1'
                    for n in range(P):
                        nsl = slice(n * PD, (n + 1) * PD)
                        pm = ppm1.tile([PD, Bc], dt.float32, tag="pm", name="pm")
                        nc.tensor.matmul(pm, cst["grow"][:, nsl], negmr,
                                         start=True, stop=False)
                        nc.tensor.matmul(pm, cst["Cg"][:, nsl], aprime,
                                         start=False, stop=True)
                        nc.scalar.activation(
                            gsb[n], pm, AF.Gelu,
                            bias=cst["be1t"][:, n:n + 1], scale=1.0,
                        )

                    for idx in range(len(iters)):
                        mg, bt = iters[idx]
                        if bt == 0:
                            w2b_cur[0] = w2tiles.pop(mg)
                        last = idx == len(iters) - 1
                        if not last:
                            po = ppc.tile([PD, 4 * PD], dt.float32, tag="po",
                                          name="po")
                            emit_po_mm(po, mg, bt, -1)
                            for k in range(P):
                                emit_po_mm(po, mg, bt, k)
                            emit_ln2_dma(po, mg, bt)
                        else:
                            # split the final iteration into two half-width
                            # POs so the tail LN2 pipeline drains sooner
                            emit_half(mg, bt, 0)
                            emit_half(mg, bt, 1)
                        if bt == NBT - 1 and mg + 2 < MG:
                            t = pw2.tile([PD, P, 4 * PD], dt.bfloat16,
                                         tag="w2b", name="w2b")
                            nc.sync.dma_start(t, w2p[mg + 2])
                            w2tiles[mg + 2] = t

    nc.compile()
    return nc


_CACHE = {}


def _get_nc(unit_ln2, zero_bias):
    key = (unit_ln2, zero_bias)
    if key not in _CACHE:
        _CACHE[key] = _build(unit_ln2, zero_bias)
    return _CACHE[key]


def _prep_in_maps(inputs):
    f32 = np.float32
    g = lambda k: np.asarray(inputs[k], f32)

    psT_full = np.asarray(g("pair_states").transpose(1, 2, 0), dtype=BF)  # [P,PD,B]
    msT_full = np.asarray(g("macro_state").T, dtype=BF)                   # [MD,B]

    W1 = g("fusion_w1")                       # (7168, 3584)
    C = np.concatenate(
        [
            g("mem_pair_vals") @ W1[:D] + g("fusion_b1")[None, :],
            g("mem_macro_vals") @ W1[D:],
        ],
        axis=0,
    )                                          # (128, 3584)
    g1 = g("fusion_ln_g")
    pw = g("pair_w")                           # (28, 256, 128)
    pwA, pwB = pw[:, :PD, :], pw[:, PD:, :]
    # W2' = W2 @ blockdiag(pwB): (3584, 28, 128)
    W2r = g("fusion_w2").reshape(D, P, PD)
    W2p = np.matmul(W2r.transpose(1, 0, 2), pwB)          # (28, 3584, 128)
    W2p = W2p.transpose(1, 0, 2).reshape(D, D)
    bp = (
        np.einsum("pc,pce->pe", g("fusion_b2").reshape(P, PD), pwB)
        + g("pair_b")
    ).reshape(1, D)

    import os
    ln2g, ln2b = g("pair_ln_g"), g("pair_ln_b")
    unit_ln2 = bool((ln2g == 1.0).all() and (ln2b == 0.0).all())
    zero_bias = bool((bp == 0.0).all())
    if os.environ.get("K_NOFAST"):
        unit_ln2 = zero_bias = False

    shared = {
        "kP": np.ascontiguousarray(
            (g("mem_pair_keys").T / (P * np.sqrt(PD))).astype(BF)),
        "kM": np.ascontiguousarray(
            (g("mem_macro_keys").T / np.sqrt(MD)).reshape(2, PD, S).astype(BF)),
        "Cg": np.ascontiguousarray((C * g1[None, :]).astype(BF)),
        "c1": np.ascontiguousarray(C.sum(axis=1, dtype=np.float64)
                                   .astype(BF).reshape(PD, 1)),
        "Gm": np.ascontiguousarray((C @ C.T).astype(BF)),
        "grow": np.ascontiguousarray(g1.reshape(1, D).astype(BF)),
        "be1t": np.ascontiguousarray(g("fusion_ln_b").reshape(P, PD).T),
        "w2p": np.ascontiguousarray(
            W2p.reshape(P, PD, MG, 4 * PD).transpose(2, 1, 0, 3).astype(BF)),
        "pwt": np.ascontiguousarray(pwA.transpose(1, 0, 2).astype(BF)),
    }
    if not zero_bias:
        shared["bprow"] = np.ascontiguousarray(bp.astype(BF))
    if not unit_ln2:
        shared["g2bc"] = np.ascontiguousarray(
            np.broadcast_to(ln2g[None], (PD, P, PD)))
        shared["b2bc"] = np.ascontiguousarray(
            np.broadcast_to(ln2b[None], (PD, P, PD)))
    in_maps = []
    for c in range(NCORES):
        m = dict(shared)
        m["psT"] = np.ascontiguousarray(psT_full[:, :, c * Bc:(c + 1) * Bc])
        m["msT"] = np.ascontiguousarray(
            msT_full[:, c * Bc:(c + 1) * Bc].reshape(2, PD, Bc))
        in_maps.append(m)
    return in_maps, unit_ln2, zero_bias


def _run(inputs, trace=False):
    in_maps, unit_ln2, zero_bias = _prep_in_maps(inputs)
    nc = _get_nc(unit_ln2, zero_bias)
    res = bass_utils.run_bass_kernel_spmd(
        nc, in_maps, core_ids=list(range(NCORES)), trace=trace
    )
    outp = np.concatenate(
        [res.results[c]["out"] for c in range(NCORES)], axis=0
    ).reshape(B, P, PD)
    return np.ascontiguousarray(outp.astype(np.float32)), res


def kernel(**inputs):
    outp, _ = _run(inputs, trace=False)
    return outp


# revision 22
# speedup vs baseline: 1.1634x; 1.1195x over previous
"""Trainium2 Bass kernel for nn_CrossPairMemory.

Sharding: data-parallel over batch across 8 NeuronCores (512 rows each),
weights replicated per core, no collectives.

Algebraic restructuring (all folds are weight-only, done host-side in fp32):
  * The fusion first Linear collapses through the associative memory read:
      h = [A_P | A_M] @ C,  C = [[vP @ W1_top + b1], [vM @ W1_bot]]
    where A_* are the (Bc, 64) attention matrices.  This removes the
    26 GFLOP/core (Bc,7168)x(7168,3584) matmul entirely.
  * LayerNorm-1 statistics come from the same algebra:
      sum_f h = c1^T a      with c1 = C.sum(axis=1)
      sum_f h^2 = a^T G a   with G = C @ C^T   (kept in fp32 on device)
    so h is never materialized pre-norm.
  * LayerNorm-1 apply is folded into the mm1 matmul: the attention matrix
    is scaled per-column by rstd, C is pre-scaled per-feature by ln_g, and
    the -mu*rstd*ln_g offset enters via a K=1 rank-1 matmul into the same
    PSUM accumulation; gelu(scale+bias) reads PSUM directly.
  * The second fusion Linear and the per-pair output Linear collapse:
      W2' = W2 @ blockdiag(pair_w[:,128:,:]),  b' = b2 @ blockdiag(..) + pair_b
    so one (Bc,3584)x(3584,3584) matmul plus a small pair_states @ pw_top
    term produces the pre-LN per-pair outputs directly, batch-major.

Input-adaptive fast paths (checked on the actual arrays, general fallback):
skip the final LN scale/shift when pair_ln_g==1 and pair_ln_b==0, and skip
the stage-C bias matmul when the folded bias is exactly zero.
"""

import sys

for _p in ("/opt/trn_rl_repo",):
    if _p not in sys.path:
        sys.path.insert(0, _p)

import numpy as np
import ml_dtypes

import concourse.bass as bass
import concourse.tile as tile
from concourse import bacc, mybir
from concourse import bass_utils

BF = ml_dtypes.bfloat16
dt = mybir.dt
AF = mybir.ActivationFunctionType
ALU = mybir.AluOpType

NCORES = 8
B, P, PD, MD, S = 4096, 28, 128, 256, 64
D = P * PD            # 3584
Bc = B // NCORES      # 512 batch rows per core
NBT = Bc // PD        # 4 batch tiles of 128
MG = 7                # mm2 column groups of 4 pairs (512 cols)
EPS = 1e-5


def _build(unit_ln1, unit_ln2, zero_bias):
    nc = bacc.Bacc(
        "TRN2", target_bir_lowering=False, debug=False, num_devices=NCORES
    )

    def din(name, shape, dty):
        return nc.dram_tensor(name, list(shape), dty, kind="ExternalInput").ap()

    psT = din("psT", (P, PD, Bc), dt.bfloat16)      # pair_states^T per pair
    qT = din("qT", (PD, Bc), dt.bfloat16)           # sum_p pair_states^T
    msT = din("msT", (2, PD, Bc), dt.bfloat16)      # macro_state^T, 2 tiles
    kP = din("kP", (PD, S), dt.bfloat16)            # pair keys^T, pre-scaled
    kM = din("kM", (2, PD, S), dt.bfloat16)         # macro keys^T, pre-scaled
    Cg = din("Cg", (PD, D), dt.bfloat16)            # C * ln1_g, slot-major
    c1 = din("c1", (PD, 1), dt.bfloat16)            # C row-sums
    Gm = din("Gm", (PD, PD), dt.float32)            # C @ C^T
    if not unit_ln1:
        g1col = din("g1col", (PD, P), dt.float32)   # ln1_g, feature-major
    be1t = din("be1t", (PD, P), dt.float32)         # ln1_b, feature-major
    w2p = din("w2p", (MG, PD, P, 4 * PD), dt.bfloat16)  # W2' blocks
    pwt = din("pwt", (PD, P, PD), dt.bfloat16)      # pair_w top half, d-major
    if not zero_bias:
        bprow = din("bprow", (1, D), dt.bfloat16)   # b2 @ pw_bot + pair_b
    if not unit_ln2:
        g2bc = din("g2bc", (PD, P, PD), dt.float32)  # pair_ln_g broadcast
        b2bc = din("b2bc", (PD, P, PD), dt.float32)  # pair_ln_b broadcast
    out = nc.dram_tensor(
        "out", [Bc, D], dt.float32, kind="ExternalOutput"
    ).ap()

    with tile.TileContext(nc) as tc:
        with (
            tc.tile_pool(name="const", bufs=1) as const,
            tc.tile_pool(name="res", bufs=1) as res,
            tc.tile_pool(name="gres", bufs=1) as gres,
            tc.tile_pool(name="w2s", bufs=2) as pw2,
        ):
            ones_col_b = const.tile([PD, 1], dt.bfloat16, tag="ocb", name="ocb")
            nc.vector.memset(ones_col_b, 1.0)
            ones_col_f = const.tile([PD, 1], dt.float32, tag="ocf", name="ocf")
            nc.vector.memset(ones_col_f, 1.0)
            ones_row_b = const.tile([1, PD], dt.bfloat16, tag="orb", name="orb")
            nc.vector.memset(ones_row_b, 1.0)
            ones_row_f = const.tile([1, PD], dt.float32, tag="orf", name="orf")
            nc.vector.memset(ones_row_f, 1.0)
            eps_t = const.tile([PD, 1], dt.float32, tag="eps", name="eps")
            nc.vector.memset(eps_t, EPS)
            warm = const.tile([PD, Bc], dt.bfloat16, tag="warm", name="warm")
            nc.vector.memset(warm, 0.0)

            cst = {}

            def cload(nm, src, shp, dty):
                t = const.tile(list(shp), dty, tag=nm, name=nm)
                nc.sync.dma_start(t, src)
                cst[nm] = t

            # DMAs in consumption order: scores path first, stage C last.
            cload("kP", kP, (PD, S), dt.bfloat16)
            cload("qT", qT, (PD, Bc), dt.bfloat16)
            kM_sb, ms_sb = [], []
            for i in range(2):
                t = const.tile([PD, S], dt.bfloat16, tag=f"kM{i}", name=f"kM{i}")
                nc.sync.dma_start(t, kM[i])
                kM_sb.append(t)
                t = const.tile([PD, Bc], dt.bfloat16, tag=f"ms{i}", name=f"ms{i}")
                nc.sync.dma_start(t, msT[i])
                ms_sb.append(t)
            cload("c1", c1, (PD, 1), dt.bfloat16)
            cload("Gm", Gm, (PD, PD), dt.bfloat16)
            if not unit_ln1:
                cload("g1col", g1col, (PD, P), dt.float32)
            cload("be1t", be1t, (PD, P), dt.float32)
            cload("Cg", Cg, (PD, D), dt.bfloat16)
            # W2' blocks and psT are needed later than the front path
            w2tiles = {}
            for mg in range(2):
                t = pw2.tile([PD, P, 4 * PD], dt.bfloat16, tag="w2b", name="w2b")
                nc.sync.dma_start(t, w2p[mg])
                w2tiles[mg] = t
            psT_sb = []
            for p in range(P):
                t = res.tile([PD, Bc], dt.bfloat16, tag=f"psT{p}", name=f"psT{p}")
                nc.sync.dma_start(t, psT[p])
                psT_sb.append(t)
            cload("pwt", pwt, (PD, P, PD), dt.bfloat16)
            if not zero_bias:
                cload("bprow", bprow, (1, D), dt.bfloat16)
            if not unit_ln2:
                cload("g2bc", g2bc, (PD, P, PD), dt.float32)
                cload("b2bc", b2bc, (PD, P, PD), dt.float32)

            # post-gelu activations, feature-major k-tiles (mm2 stationary)
            gsb = [
                gres.tile([PD, Bc], dt.bfloat16, tag=f"g{n}", name=f"g{n}")
                for n in range(P)
            ]

            # ---------------- front: memory read + LN1 + gelu ----------
            with tc.tile_pool(name="fr", bufs=1) as fr:
                abPM = fr.tile([PD, Bc], dt.bfloat16, tag="abPM", name="abPM")
                aprime = fr.tile([PD, Bc], dt.bfloat16, tag="apr", name="apr")
                negmr = fr.tile([1, Bc], dt.bfloat16, tag="negmr", name="negmr")

                with (
                    tc.tile_pool(name="psWm", bufs=2, space="PSUM") as ppwm,
                    tc.tile_pool(name="psSp", bufs=2, space="PSUM") as ppsp,
                    tc.tile_pool(name="psBc", bufs=2, space="PSUM") as ppbc,
                    tc.tile_pool(name="psRw", bufs=2, space="PSUM") as pprw,
                ):
                    # spin the PE p-state up while input DMAs stream
                    for _ in range(8):
                        wps = ppwm.tile([PD, Bc], dt.float32, tag="wps",
                                        name="wps")
                        nc.tensor.matmul(wps, warm[:, 0:PD], warm,
                                         start=True, stop=True)

                    spP = ppsp.tile([S, Bc], dt.float32, tag="sp", name="spP")
                    nc.tensor.matmul(spP, cst["kP"], cst["qT"],
                                     start=True, stop=True)
                    ebP = fr.tile([S, Bc], dt.bfloat16, tag="ebP", name="ebP")
                    nc.scalar.activation(ebP, spP, AF.Exp)
                    spM = ppsp.tile([S, Bc], dt.float32, tag="sp", name="spM")
                    nc.tensor.matmul(spM, kM_sb[0], ms_sb[0],
                                     start=True, stop=False)
                    nc.tensor.matmul(spM, kM_sb[1], ms_sb[1],
                                     start=False, stop=True)
                    ebM = fr.tile([S, Bc], dt.bfloat16, tag="ebM", name="ebM")
                    nc.scalar.activation(ebM, spM, AF.Exp)
                    denP = pprw.tile([1, Bc], dt.float32, tag="den", name="denP")
                    nc.tensor.matmul(denP, ones_col_b[0:S, :], ebP,
                                     start=True, stop=True)
                    denM = pprw.tile([1, Bc], dt.float32, tag="den", name="denM")
                    nc.tensor.matmul(denM, ones_col_b[0:S, :], ebM,
                                     start=True, stop=True)
                    rrP = fr.tile([1, Bc], dt.float32, tag="rrP", name="rrP")
                    nc.vector.reciprocal(rrP, denP)
                    rrM = fr.tile([1, Bc], dt.float32, tag="rrM", name="rrM")
                    nc.vector.reciprocal(rrM, denM)
                    rbcP = ppbc.tile([S, Bc], dt.float32, tag="rbc", name="rbcP")
                    nc.tensor.matmul(rbcP, ones_row_f[:, 0:S], rrP,
                                     start=True, stop=True)
                    rbcM = ppbc.tile([S, Bc], dt.float32, tag="rbc", name="rbcM")
                    nc.tensor.matmul(rbcM, ones_row_f[:, 0:S], rrM,
                                     start=True, stop=True)
                    nc.vector.tensor_mul(abPM[0:S, :], ebP, rbcP)
                    nc.vector.tensor_mul(abPM[S:2 * S, :], ebM, rbcM)

                with tc.tile_pool(name="psSt", bufs=1, space="PSUM") as ppst:
                    murow = ppst.tile([1, Bc], dt.float32, tag="mu", name="mu")
                    nc.tensor.matmul(murow, cst["c1"], abPM, start=True, stop=True)
                    Gt = ppst.tile([PD, Bc], dt.float32, tag="Gt", name="Gt")
                    nc.tensor.matmul(Gt, cst["Gm"], abPM, start=True, stop=True)
                    qq = fr.tile([PD, Bc], dt.bfloat16, tag="qq", name="qq")
                    nc.vector.tensor_mul(qq, abPM, Gt)
                    sqrow = ppst.tile([1, Bc], dt.float32, tag="sq", name="sq")
                    nc.tensor.matmul(sqrow, ones_col_b, qq, start=True, stop=True)

                    m = fr.tile([1, Bc], dt.float32, tag="m", name="m")
                    nc.vector.tensor_scalar_mul(m, murow, 1.0 / D)
                    m2 = fr.tile([1, Bc], dt.float32, tag="m2", name="m2")
                    nc.vector.tensor_mul(m2, m, m)
                    var = fr.tile([1, Bc], dt.float32, tag="var", name="var")
                    nc.vector.scalar_tensor_tensor(
                        var, sqrow, 1.0 / D, m2, op0=ALU.mult, op1=ALU.subtract
                    )
                    sd = fr.tile([1, Bc], dt.float32, tag="sd", name="sd")
                    nc.scalar.activation(sd, var, AF.Sqrt,
                                         bias=eps_t[0:1, :], scale=1.0)
                    rstd = fr.tile([1, Bc], dt.float32, tag="rstd", name="rstd")
                    nc.vector.reciprocal(rstd, sd)
                    # negmr = -mu * rstd (bf16 row, K=1 matmul operand)
                    nc.vector.scalar_tensor_tensor(
                        negmr, m, -1.0, rstd, op0=ALU.mult, op1=ALU.mult
                    )
                    rstd_bc = ppst.tile([PD, Bc], dt.float32, tag="rbc2",
                                        name="rbc2")
                    nc.tensor.matmul(rstd_bc, ones_row_f, rstd,
                                     start=True, stop=True)
                    nc.vector.tensor_mul(aprime, abPM, rstd_bc)

                # ---- mm1' + stage BC, software-pipelined ----
                # mm1' per tile n: C-matmul, then the -mu*rstd*g offset is a
                # DVE add of a broadcast PSUM tile (off the PE critical path),
                # then gelu reads PSUM.  The first PRE stage-C accumulations
                # interleave k=n matmuls two steps behind gsb production, so
                # stage C starts while the front is still finishing.
                iters = [(mg, bt) for mg in range(MG) for bt in range(NBT)]
                with (
                    tc.tile_pool(name="psM1", bufs=2, space="PSUM") as ppm1,
                    tc.tile_pool(name="psOf", bufs=1, space="PSUM") as ppof,
                    tc.tile_pool(name="pm2p", bufs=2) as pm2p,
                    tc.tile_pool(name="yo", bufs=3) as pyo,
                    tc.tile_pool(name="sc", bufs=4) as psc,
                    tc.tile_pool(name="psC", bufs=3, space="PSUM") as ppc,
                    tc.tile_pool(name="psCh", bufs=2, space="PSUM") as ppch,
                ):

                    def emit_po_mm(po, mg, bt, k):
                        bs = slice(bt * PD, (bt + 1) * PD)
                        first = (k == 0) if zero_bias else (k == -1)
                        if k == -1:
                            if not zero_bias:
                                nc.tensor.matmul(
                                    po, ones_row_b,
                                    cst["bprow"][:, mg * 4 * PD:
                                                 (mg + 1) * 4 * PD],
                                    start=True, stop=False)
                            return
                        nc.tensor.matmul(
                            po, gsb[k][:, bs], w2b_cur[0][:, k, :],
                            start=first, stop=(k == P - 1),
                            skip_group_check=not first,
                        )
                        if k == 0:
                            for s in range(4):
                                pidx = 4 * mg + s
                                nc.tensor.matmul(
                                    po[:, s * PD:(s + 1) * PD],
                                    psT_sb[pidx][:, bs],
                                    cst["pwt"][:, pidx, :],
                                    start=False, stop=False,
                                    skip_group_check=True,
                                )

                    def emit_ln2_dma(po, mg, bt):
                        bs = slice(bt * PD, (bt + 1) * PD)
                        mgsl = slice(mg * 4 * PD, (mg + 1) * 4 * PD)
                        y4 = pyo.tile([PD, 4 * PD], dt.float32, tag="y4",
                                      name="y4")
                        mva = psc.tile([PD, 8], dt.float32, tag="mv", name="mv")
                        rst4 = psc.tile([PD, 4], dt.float32, tag="rst4",
                                        name="rst4")
                        for s in range(4):
                            st6 = psc.tile([PD, 6], dt.float32, tag="st6",
                                           name="st6")
                            nc.vector.bn_stats(st6, po[:, s * PD:(s + 1) * PD])
                            nc.vector.bn_aggr(mva[:, 2 * s:2 * s + 2], st6)
                            sd2 = psc.tile([PD, 1], dt.float32, tag="sd2",
                                           name="sd2")
                            nc.scalar.activation(
                                sd2, mva[:, 2 * s + 1:2 * s + 2], AF.Sqrt,
                                bias=eps_t, scale=1.0)
                            nc.vector.reciprocal(rst4[:, s:s + 1], sd2)
                        for s in range(4):
                            pidx = 4 * mg + s
                            ssl = slice(s * PD, (s + 1) * PD)
                            if unit_ln2:
                                nc.vector.tensor_scalar(
                                    y4[:, ssl], po[:, ssl],
                                    mva[:, 2 * s:2 * s + 1], rst4[:, s:s + 1],
                                    op0=ALU.subtract, op1=ALU.mult,
                                )
                            else:
                                tn = psc.tile([PD, PD], dt.float32, tag="tn",
                                              name="tn")
                                nc.vector.tensor_scalar(
                                    tn, po[:, ssl],
                                    mva[:, 2 * s:2 * s + 1], rst4[:, s:s + 1],
                                    op0=ALU.subtract, op1=ALU.mult,
                                )
                                tg = psc.tile([PD, PD], dt.float32, tag="tg",
                                              name="tg")
                                nc.vector.tensor_mul(
                                    tg, tn, cst["g2bc"][:, pidx, :])
                                nc.vector.tensor_add(
                                    y4[:, ssl], tg, cst["b2bc"][:, pidx, :])
                        nc.sync.dma_start(out[bs, mgsl], y4)

                    def emit_half(mg, bt, h):
                        bs = slice(bt * PD, (bt + 1) * PD)
                        hsl = slice(h * 2 * PD, (h + 1) * 2 * PD)
                        po = ppch.tile([PD, 2 * PD], dt.float32, tag="poh",
                                       name=f"poh{h}")
                        if not zero_bias:
                            nc.tensor.matmul(
                                po, ones_row_b,
                                cst["bprow"][:, mg * 4 * PD + h * 2 * PD:
                                             mg * 4 * PD + (h + 1) * 2 * PD],
                                start=True, stop=False)
                        for k in range(P):
                            first = zero_bias and k == 0
                            nc.tensor.matmul(
                                po, gsb[k][:, bs], w2b_cur[0][:, k, hsl],
                                start=first, stop=(k == P - 1),
                                skip_group_check=not first)
                            if k == 0:
                                for s in range(2):
                                    pidx = 4 * mg + 2 * h + s
                                    nc.tensor.matmul(
                                        po[:, s * PD:(s + 1) * PD],
                                        psT_sb[pidx][:, bs],
                                        cst["pwt"][:, pidx, :],
                                        start=False, stop=False,
                                        skip_group_check=True)
                        y2 = pyo.tile([PD, 2 * PD], dt.float32, tag="y2",
                                      name=f"y2{h}")
                        mv2 = psc.tile([PD, 4], dt.float32, tag="mv2",
                                       name="mv2")
                        rs2 = psc.tile([PD, 2], dt.float32, tag="rs2",
                                       name="rs2")
                        for s in range(2):
                            pidx = 4 * mg + 2 * h + s
                            ssl = slice(s * PD, (s + 1) * PD)
                            st6 = psc.tile([PD, 6], dt.float32, tag="st6",
                                           name="st6")
                            nc.vector.bn_stats(st6, po[:, ssl])
                            nc.vector.bn_aggr(mv2[:, 2 * s:2 * s + 2], st6)
                            sd2 = psc.tile([PD, 1], dt.float32, tag="sd2",
                                           name="sd2")
                            nc.scalar.activation(
                                sd2, mv2[:, 2 * s + 1:2 * s + 2], AF.Sqrt,
                                bias=eps_t, scale=1.0)
                            nc.vector.reciprocal(rs2[:, s:s + 1], sd2)
                            if unit_ln2:
                                nc.vector.tensor_scalar(
                                    y2[:, ssl], po[:, ssl],
                                    mv2[:, 2 * s:2 * s + 1], rs2[:, s:s + 1],
                                    op0=ALU.subtract, op1=ALU.mult)
                            else:
                                tn = psc.tile([PD, PD], dt.float32, tag="tn",
                                              name="tn")
                                nc.vector.tensor_scalar(
                                    tn, po[:, ssl],
                                    mv2[:, 2 * s:2 * s + 1], rs2[:, s:s + 1],
                                    op0=ALU.subtract, op1=ALU.mult)
                                tg = psc.tile([PD, PD], dt.float32, tag="tg",
                                              name="tg")
                                nc.vector.tensor_mul(
                                    tg, tn, cst["g2bc"][:, pidx, :])
                                nc.vector.tensor_add(
                                    y2[:, ssl], tg, cst["b2bc"][:, pidx, :])
                        nc.sync.dma_start(
                            out[bs, mg * 4 * PD + h * 2 * PD:
                                mg * 4 * PD + (h + 1) * 2 * PD], y2)

                    w2b_cur = [None]
                    # -mu*rstd*g broadcast, added on DVE off the PE path.
                    # (general path note: ln1_g premultiplies Cg; the rank-1
                    # offset uses the g row via the K=1 matmul broadcast)
                    ofs_ps = ppof.tile([PD, Bc], dt.float32, tag="ofs",
                                       name="ofs_ps")
                    nc.tensor.matmul(ofs_ps, ones_row_b, negmr,
                                     start=True, stop=True)
                    ofs = pm2p.tile([PD, Bc], dt.float32, tag="ofs_sb",
                                    name="ofs_sb")
                    nc.scalar.activation(ofs, ofs_ps, AF.Copy)
                    # mm1'
                    for n in range(P):
                        nsl = slice(n * PD, (n + 1) * PD)
                        pm = ppm1.tile([PD, Bc], dt.float32, tag="pm", name="pm")
                        nc.tensor.matmul(pm, cst["Cg"][:, nsl], aprime,
                                         start=True, stop=True)
                        pm2 = pm2p.tile([PD, Bc], dt.float32, tag="pm2",
                                        name="pm2")
                        if unit_ln1:
                            nc.vector.tensor_add(pm2, pm, ofs)
                        else:
                            nc.vector.scalar_tensor_tensor(
                                pm2, ofs, cst["g1col"][:, n:n + 1], pm,
                                op0=ALU.mult, op1=ALU.add)
                        nc.scalar.activation(
                            gsb[n], pm2, AF.Gelu,
                            bias=cst["be1t"][:, n:n + 1], scale=1.0,
                        )

                    for idx in range(len(iters)):
                        mg, bt = iters[idx]
                        if bt == 0:
                            w2b_cur[0] = w2tiles.pop(mg)
                        last = idx == len(iters) - 1
                        if not last:
                            po = ppc.tile([PD, 4 * PD], dt.float32, tag="po",
                                          name="po")
                            emit_po_mm(po, mg, bt, -1)
                            for k in range(P):
                                emit_po_mm(po, mg, bt, k)
                            emit_ln2_dma(po, mg, bt)
                        else:
                            # split the final iteration into two half-width
                            # POs so the tail LN2 pipeline drains sooner
                            emit_half(mg, bt, 0)
                            emit_half(mg, bt, 1)
                        if bt == NBT - 1 and mg + 2 < MG:
                            t = pw2.tile([PD, P, 4 * PD], dt.bfloat16,
                                         tag="w2b", name="w2b")
                            nc.sync.dma_start(t, w2p[mg + 2])
                            w2tiles[mg + 2] = t

    nc.compile()
    return nc


_CACHE = {}


def _get_nc(unit_ln1, unit_ln2, zero_bias):
    key = (unit_ln1, unit_ln2, zero_bias)
    if key not in _CACHE:
        _CACHE[key] = _build(unit_ln1, unit_ln2, zero_bias)
    return _CACHE[key]


def _prep_in_maps(inputs):
    f32 = np.float32
    g = lambda k: np.asarray(inputs[k], f32)

    psT_full = np.asarray(g("pair_states").transpose(1, 2, 0), dtype=BF)  # [P,PD,B]
    msT_full = np.asarray(g("macro_state").T, dtype=BF)                   # [MD,B]
    qT_full = np.asarray(g("pair_states").sum(axis=1).T, dtype=BF)        # [PD,B]

    W1 = g("fusion_w1")                       # (7168, 3584)
    C = np.concatenate(
        [
            g("mem_pair_vals") @ W1[:D] + g("fusion_b1")[None, :],
            g("mem_macro_vals") @ W1[D:],
        ],
        axis=0,
    )                                          # (128, 3584)
    g1 = g("fusion_ln_g")
    pw = g("pair_w")                           # (28, 256, 128)
    pwA, pwB = pw[:, :PD, :], pw[:, PD:, :]
    # W2' = W2 @ blockdiag(pwB): (3584, 28, 128)
    W2r = g("fusion_w2").reshape(D, P, PD)
    W2p = np.matmul(W2r.transpose(1, 0, 2), pwB)          # (28, 3584, 128)
    W2p = W2p.transpose(1, 0, 2).reshape(D, D)
    bp = (
        np.einsum("pc,pce->pe", g("fusion_b2").reshape(P, PD), pwB)
        + g("pair_b")
    ).reshape(1, D)

    import os
    ln2g, ln2b = g("pair_ln_g"), g("pair_ln_b")
    unit_ln1 = bool((g1 == 1.0).all())
    unit_ln2 = bool((ln2g == 1.0).all() and (ln2b == 0.0).all())
    zero_bias = bool((bp == 0.0).all())
    if os.environ.get("K_NOFAST"):
        unit_ln1 = unit_ln2 = zero_bias = False

    shared = {
        "kP": np.ascontiguousarray(
            (g("mem_pair_keys").T / (P * np.sqrt(PD))).astype(BF)),
        "kM": np.ascontiguousarray(
            (g("mem_macro_keys").T / np.sqrt(MD)).reshape(2, PD, S).astype(BF)),
        "Cg": np.ascontiguousarray((C * g1[None, :]).astype(BF)),
        "c1": np.ascontiguousarray(C.sum(axis=1, dtype=np.float64)
                                   .astype(BF).reshape(PD, 1)),
        "Gm": np.ascontiguousarray((C @ C.T).astype(BF)),
        "be1t": np.ascontiguousarray(g("fusion_ln_b").reshape(P, PD).T),
        "w2p": np.ascontiguousarray(
            W2p.reshape(P, PD, MG, 4 * PD).transpose(2, 1, 0, 3).astype(BF)),
        "pwt": np.ascontiguousarray(pwA.transpose(1, 0, 2).astype(BF)),
    }
    if not unit_ln1:
        shared["g1col"] = np.ascontiguousarray(g1.reshape(P, PD).T.astype(f32))
    if not zero_bias:
        shared["bprow"] = np.ascontiguousarray(bp.astype(BF))
    if not unit_ln2:
        shared["g2bc"] = np.ascontiguousarray(
            np.broadcast_to(ln2g[None], (PD, P, PD)))
        shared["b2bc"] = np.ascontiguousarray(
            np.broadcast_to(ln2b[None], (PD, P, PD)))
    in_maps = []
    for c in range(NCORES):
        m = dict(shared)
        m["psT"] = np.ascontiguousarray(psT_full[:, :, c * Bc:(c + 1) * Bc])
        m["msT"] = np.ascontiguousarray(
            msT_full[:, c * Bc:(c + 1) * Bc].reshape(2, PD, Bc))
        m["qT"] = np.ascontiguousarray(qT_full[:, c * Bc:(c + 1) * Bc])
        in_maps.append(m)
    return in_maps, unit_ln1, unit_ln2, zero_bias


def _run(inputs, trace=False):
    in_maps, unit_ln1, unit_ln2, zero_bias = _prep_in_maps(inputs)
    nc = _get_nc(unit_ln1, unit_ln2, zero_bias)
    res = bass_utils.run_bass_kernel_spmd(
        nc, in_maps, core_ids=list(range(NCORES)), trace=trace
    )
    outp = np.concatenate(
        [res.results[c]["out"] for c in range(NCORES)], axis=0
    ).reshape(B, P, PD)
    return np.ascontiguousarray(outp.astype(np.float32)), res


def kernel(**inputs):
    outp, _ = _run(inputs, trace=False)
    return outp
